# revision 1
# baseline (speedup 1.0000x reference)
"""Multi-head self-attention (L=2048, N=4, E=1024, h=16) on 8 NeuronCores.

Sharding: core c handles batch n = c//2 and heads [8*(c%2), 8*(c%2)+8).
Each core computes q/k/v projections for its (n, head-block), attention,
and a partial out-projection (columns of out_proj for its heads).
Host sums the two partials per batch n and adds out_bias.

PE strategy (all operands bf16, accumulation fp32 in PSUM):
- q/k/v projections: K=128 matmuls over 8 E-tiles.
- QK^T: row-packed pairs (two K=64 matmuls on row groups 0-1/2-3 run
  concurrently in the PE array).
- softmax: no max-subtraction (scores are small by construction);
  denominators via M=1 ones-matmuls, 4 heads col-packed per 32-strips;
  reciprocal on DVE, broadcast via gpsimd partition_broadcast.
- attn @ V: col-packed pairs (M=64 at tile_position (0,0)/(0,64)).
- out projection: K=128 over 4 stacked head-pair tiles.
"""

from contextlib import ExitStack

import ml_dtypes
import numpy as np

import concourse.bacc as bacc
import concourse.mybir as mybir
import concourse.tile as tile
from concourse.bass_utils import run_bass_kernel_spmd

L, N, E, H, D = 2048, 4, 1024, 16, 64
SCALE = D**-0.5
IL = 512  # inner dims per core (8 heads * 64)
P = 128
F32 = mybir.dt.float32
BF16 = mybir.dt.bfloat16
EXP = mybir.ActivationFunctionType.Exp

_built = None


def build(dbg=False, reps=1, loop_reps=1):
    nc = bacc.Bacc("TRN2", target_bir_lowering=False, debug=False, num_devices=8)

    qt_d = nc.dram_tensor("qt", [E, L], BF16, kind="ExternalInput")
    wq_d = nc.dram_tensor("wq", [E, IL], BF16, kind="ExternalInput")
    wk_d = nc.dram_tensor("wk", [E, IL], BF16, kind="ExternalInput")
    wv_d = nc.dram_tensor("wv", [E, IL], BF16, kind="ExternalInput")
    bq_d = nc.dram_tensor("bq", [4, P], F32, kind="ExternalInput")
    bk_d = nc.dram_tensor("bk", [4, P], F32, kind="ExternalInput")
    bvb_d = nc.dram_tensor("bvb", [P, IL], F32, kind="ExternalInput")
    opt_d = nc.dram_tensor("opt", [IL, E], BF16, kind="ExternalInput")
    out_d = nc.dram_tensor("out", [L, E], F32, kind="ExternalOutput")

    with tile.TileContext(nc) as tc:
      lctx = tc.For_i(0, loop_reps, 1) if loop_reps > 1 else None
      if lctx is not None:
          lctx.__enter__()
      for _rep in range(reps):
        est = ExitStack()
        persist = est.enter_context(tc.tile_pool(name="persist", bufs=1))

        ones_col = persist.tile([P, 1], BF16, name="ones_col")
        nc.vector.memset(ones_col, 1.0)

        bq_sb = persist.tile([P, 4], F32, name="bq_sb")
        bk_sb = persist.tile([P, 4], F32, name="bk_sb")
        for m in range(4):
            nc.sync.dma_start(out=bq_sb[:, m : m + 1], in_=bq_d[m, :, None])
            nc.sync.dma_start(out=bk_sb[:, m : m + 1], in_=bk_d[m, :, None])
        bvb_sb = persist.tile([P, IL], F32, name="bvb_sb")
        nc.sync.dma_start(out=bvb_sb, in_=bvb_d[:, :])

        qT = [persist.tile([P, L], BF16, name=f"qT{m}") for m in range(4)]
        kT = [persist.tile([P, L], BF16, name=f"kT{m}") for m in range(4)]
        vv = [persist.tile([P, IL], BF16, name=f"v{t}") for t in range(16)]
        aoT = [persist.tile([P, L], BF16, name=f"aoT{m}") for m in range(4)]
        opt_sb = [persist.tile([P, E], BF16, name=f"opt{k}") for k in range(4)]
        for k in range(4):
            nc.sync.dma_start(out=opt_sb[k], in_=opt_d[k * P : (k + 1) * P, :])

        # ---------------- phase 1 setup: streaming inputs ----------------
        ph_all = est.enter_context(ExitStack())
        qt_pool = ph_all.enter_context(tc.tile_pool(name="qt_pool", bufs=8))
        w_pool = ph_all.enter_context(tc.tile_pool(name="w_pool", bufs=8))
        qt_sb = [qt_pool.tile([P, L], BF16, tag="qt", name=f"qtsb{t}") for t in range(8)]
        wq_sb = [w_pool.tile([P, IL], BF16, tag="wq", name=f"wq{t}") for t in range(8)]
        wk_sb = [w_pool.tile([P, IL], BF16, tag="wk", name=f"wk{t}") for t in range(8)]
        wv_sb = [w_pool.tile([P, IL], BF16, tag="wv", name=f"wv{t}") for t in range(8)]
        for t in range(8):
            nc.sync.dma_start(out=qt_sb[t], in_=qt_d[t * P : (t + 1) * P, :])
            nc.sync.dma_start(out=wq_sb[t], in_=wq_d[t * P : (t + 1) * P, :])
            nc.sync.dma_start(out=wk_sb[t], in_=wk_d[t * P : (t + 1) * P, :])
        for t in range(8):
            nc.sync.dma_start(out=wv_sb[t], in_=wv_d[t * P : (t + 1) * P, :])

        # q/k projection Mtiles 0..1 up front (4-bank psum pool, then closed)
        with tc.tile_pool(name="qk_ps", bufs=2, space="PSUM") as qk_ps:
            def qk_mtile(m):
                for half in range(2):
                    for w_sb, bias_sb, dest, nm in (
                        (wq_sb, bq_sb, qT, "q"),
                        (wk_sb, bk_sb, kT, "k"),
                    ):
                        ps = qk_ps.tile(
                            [P, L // 2], F32, tag="qkps", name=f"ps{nm}{m}{half}"
                        )
                        for t in range(8):
                            for c in range(2):
                                nc.tensor.matmul(
                                    ps[:, c * 512 : (c + 1) * 512],
                                    w_sb[t][:, m * P : (m + 1) * P],
                                    qt_sb[t][
                                        :,
                                        (2 * half + c) * 512 : (2 * half + c + 1) * 512,
                                    ],
                                    start=(t == 0),
                                    stop=(t == 7),
                                )
                        nc.vector.tensor_scalar_add(
                            out=dest[m][:, half * 1024 : (half + 1) * 1024],
                            in0=ps,
                            scalar1=bias_sb[:, m : m + 1],
                        )

            qk_mtile(0)
            qk_mtile(1)

        # ---------------- phase 2: attention with interleaved fillers ----------
        with ExitStack() as ph2:
            at_pools = [
                ph2.enter_context(tc.tile_pool(name=f"at{i}", bufs=3)) for i in (0, 1)
            ]
            small = ph2.enter_context(tc.tile_pool(name="small", bufs=4))
            osb = ph2.enter_context(tc.tile_pool(name="osb", bufs=3))
            st_ps = [
                ph2.enter_context(tc.tile_pool(name=f"st{i}", bufs=1, space="PSUM"))
                for i in (0, 1)
            ]
            pv_ps = [
                ph2.enter_context(tc.tile_pool(name=f"pv{i}", bufs=1, space="PSUM"))
                for i in (0, 1)
            ]
            den_ps = ph2.enter_context(tc.tile_pool(name="den", bufs=1, space="PSUM"))

            fillers = []  # deque of thunks, each ~0.5-2us of PE work

            def make_qk23_fillers(ps_pool):
                for m in (2, 3):
                    for w_sb, bias_sb, dest, nm in (
                        (wq_sb, bq_sb, qT, "q"),
                        (wk_sb, bk_sb, kT, "k"),
                    ):
                        for ch in range(4):
                            def thunk(m=m, w_sb=w_sb, bias_sb=bias_sb, dest=dest,
                                      nm=nm, ch=ch):
                                ps = ps_pool.tile(
                                    [P, 512], F32, tag="qk2",
                                    name=f"p2{nm}{m}{ch}",
                                )
                                for t in range(8):
                                    nc.tensor.matmul(
                                        ps,
                                        w_sb[t][:, m * P : (m + 1) * P],
                                        qt_sb[t][:, ch * 512 : (ch + 1) * 512],
                                        start=(t == 0),
                                        stop=(t == 7),
                                    )
                                nc.vector.tensor_scalar_add(
                                    out=dest[m][:, ch * 512 : (ch + 1) * 512],
                                    in0=ps,
                                    scalar1=bias_sb[:, m : m + 1],
                                )
                            fillers.append(thunk)

            def make_outproj_fillers(ps_pool, lts):
                for lt in lts:
                    for c in (0, 1):
                        def thunk(lt=lt, c=c):
                            ps = ps_pool.tile(
                                [P, 512], F32, tag="ops", name=f"ops{lt}{c}"
                            )
                            for k in range(4):
                                nc.tensor.matmul(
                                    ps,
                                    aoT[k][:, lt * P : (lt + 1) * P],
                                    opt_sb[k][:, c * 512 : (c + 1) * 512],
                                    start=(k == 0),
                                    stop=(k == 3),
                                )
                            ob = osb.tile([P, 512], F32, tag="ob", name=f"ob{lt}{c}")
                            nc.vector.tensor_copy(out=ob, in_=ps)
                            nc.sync.dma_start(
                                out=out_d[lt * P : (lt + 1) * P, c * 512 : (c + 1) * 512],
                                in_=ob,
                            )
                        fillers.append(thunk)

            def attn_chunk(rnd, lq, v_interleave, fill_budget):
                lanes = (2 * rnd, 2 * rnd + 1)
                lqs = slice(lq * 512, (lq + 1) * 512)
                den_t = den_ps.tile([P, 512], F32, tag="den", name=f"den_{rnd}_{lq}")
                pv_t = {}
                for i, p in enumerate(lanes):
                    pv_t[p] = pv_ps[i].tile(
                        [P, 512], F32, tag="pv", name=f"pv_{p}_{lq}"
                    )

                def pv_den_step(lk, ats):
                    for i, p in enumerate(lanes):
                        for j in (0, 1):
                            nc.tensor.matmul(
                                pv_t[p][64 * j : 64 * j + 64, :],
                                vv[lk][:, P * p + 64 * j : P * p + 64 * j + 64],
                                ats[i][:, j, :],
                                start=(lk == 0),
                                stop=(lk == 15),
                            )
                    for i, p in enumerate(lanes):
                        for j in (0, 1):
                            r0 = 64 * i + 32 * j
                            nc.tensor.matmul(
                                den_t[r0 : r0 + 1, :],
                                ones_col,
                                ats[i][:, j, :],
                                start=(lk == 0),
                                stop=(lk == 15),
                                tile_position=(0, r0),
                            )

                prev = None
                for lk in range(16):
                    lks = slice(lk * P, (lk + 1) * P)
                    ats = []
                    for i, p in enumerate(lanes):
                        st = st_ps[i].tile(
                            [P, 2, 512], F32, tag="st", name=f"st_{p}_{lq}_{lk}"
                        )
                        for j in (0, 1):
                            nc.tensor.matmul(
                                st[:, j, :],
                                kT[p][64 * j : 64 * j + 64, lks],
                                qT[p][64 * j : 64 * j + 64, lqs],
                                start=True,
                                stop=True,
                            )
                        at = at_pools[i].tile(
                            [P, 2, 512], BF16, tag="at", name=f"at_{p}_{lq}_{lk}"
                        )
                        nc.scalar.activation(out=at, in_=st, func=EXP)
                        ats.append(at)
                    if v_interleave is not None:
                        v_interleave(lk)
                    for _ in range(fill_budget):
                        if fillers:
                            fillers.pop(0)()
                    if prev is not None:
                        pv_den_step(lk - 1, prev)
                    prev = ats
                pv_den_step(15, prev)

                for i, p in enumerate(lanes):
                    bcs = small.tile(
                        [P, 2, 512], F32, tag="bcs", name=f"bcs_{p}_{lq}", bufs=2
                    )
                    rc = small.tile(
                        [1, 2, 512], F32, tag="rc", name=f"rc_{p}_{lq}", bufs=2
                    )
                    for j in (0, 1):
                        r0 = 64 * i + 32 * j
                        nc.vector.reciprocal(out=rc[:, j, :], in_=den_t[r0 : r0 + 1, :])
                    nc.gpsimd.partition_broadcast(bcs, rc)
                    for j in (0, 1):
                        nc.vector.tensor_mul(
                            out=aoT[p][64 * j : 64 * j + 64, lqs],
                            in0=pv_t[p][64 * j : 64 * j + 64, :],
                            in1=bcs[64 * j : 64 * j + 64, j, :],
                        )

            # round 0, chunk 0: v projection rides inside the lk loop
            with tc.tile_pool(name="v_ps", bufs=1, space="PSUM") as v_ps:
                def v_interleave(lk):
                    ps = v_ps.tile([P, IL], F32, tag="vps", name=f"psv{lk}")
                    for t in range(8):
                        nc.tensor.matmul(
                            ps,
                            qt_sb[t][:, lk * P : (lk + 1) * P],
                            wv_sb[t],
                            start=(t == 0),
                            stop=(t == 7),
                        )
                    nc.vector.tensor_add(out=vv[lk], in0=ps, in1=bvb_sb)

                attn_chunk(0, 0, v_interleave, 0)

            # round 0, chunks 1-3: q/k Mtiles 2,3 fill PE idle
            with tc.tile_pool(name="qk2_ps", bufs=1, space="PSUM") as qk2_ps:
                make_qk23_fillers(qk2_ps)
                for lq in range(1, 4):
                    attn_chunk(0, lq, None, 1)
                while fillers:
                    fillers.pop(0)()

            # round 1: out-projection of previous chunks fills PE idle
            with tc.tile_pool(name="o_ps", bufs=1, space="PSUM") as o_ps:
                for lq in range(4):
                    if lq >= 1:
                        make_outproj_fillers(o_ps, range(4 * (lq - 1), 4 * lq))
                    attn_chunk(1, lq, None, 1)
                make_outproj_fillers(o_ps, range(12, 16))
                while fillers:
                    fillers.pop(0)()

        est.close()

      if lctx is not None:
          lctx.__exit__(None, None, None)

    nc.compile()
    return nc


def _prep_inputs(query, qkv_proj, qkv_bias, out_proj):
    """Per-core input shards (host-side)."""
    query = np.asarray(query, dtype=np.float32)
    qkv_proj = np.asarray(qkv_proj, dtype=np.float32)
    qkv_bias = np.asarray(qkv_bias, dtype=np.float32)
    W3 = qkv_proj.reshape(E, 3, E)  # [i, c, e], row f = 3*i + c
    b3 = qkv_bias.reshape(E, 3)
    bf = ml_dtypes.bfloat16
    maps = []
    for c in range(8):
        n, half = c // 2, c % 2
        isl = slice(IL * half, IL * half + IL)
        maps.append(
            {
                "qt": np.ascontiguousarray(query[:, n, :].T).astype(bf),
                "wq": np.ascontiguousarray(W3[isl, 0, :].T * SCALE).astype(bf),
                "wk": np.ascontiguousarray(W3[isl, 1, :].T).astype(bf),
                "wv": np.ascontiguousarray(W3[isl, 2, :].T).astype(bf),
                "bq": np.ascontiguousarray((b3[isl, 0] * SCALE).reshape(4, P)),
                "bk": np.ascontiguousarray(b3[isl, 1].reshape(4, P)),
                "bvb": np.ascontiguousarray(np.broadcast_to(b3[isl, 2], (P, IL))),
                "opt": np.ascontiguousarray(out_proj[:, isl].T).astype(bf),
            }
        )
    return maps


def kernel(query, qkv_proj, qkv_bias, out_proj, out_bias, **run_kwargs):
    global _built
    out_proj = np.asarray(out_proj, dtype=np.float32)
    out_bias = np.asarray(out_bias, dtype=np.float32)
    if _built is None:
        _built = build()
    in_maps = _prep_inputs(query, qkv_proj, qkv_bias, out_proj)
    res = run_bass_kernel_spmd(_built, in_maps, core_ids=list(range(8)), **run_kwargs)
    parts = [r["out"] for r in res.results]
    out = np.empty((L, N, E), dtype=np.float32)
    for n in range(N):
        out[:, n, :] = parts[2 * n] + parts[2 * n + 1] + out_bias
    kernel.last_result = res
    return out

